# revision 26
# baseline (speedup 1.0000x reference)
"""Trainium2 Bass kernel: causal self-attention (GQA + RoPE) for
B=1, T=2048, C=2048, H=16 query heads, HKV=4 KV heads, D=128.

Sharding: tensor-parallel over heads across 8 NeuronCores. Core m computes
query heads {2m, 2m+1} and the single KV head (m//2) those heads attend to,
plus the o_proj partial product for its 256 input columns. The host sums the
8 partial outputs (the TP all-reduce).

v2 (vs the fp32r baseline at ~219us):
  * All matmul operands are bf16 (tolerance is 2e-2; baseline err 1.9e-3).
    Halves HBM traffic and SBUF footprint, and DVE elementwise ops hit the
    2x/4x 16-bit perf modes.
  * The 45us DMA lead-in is gone: weights stream in [128,256] c-chunks on
    the GpSimd queue interleaved with [128,2048] x c-chunks on the Sync
    queue, so the first projection matmul only waits for chunk 0 (~4us).
  * The 80 ones-matmul softmax-denominator reductions (~19us of PE) are
    replaced by bf16 DVE accumulations of the exp tiles plus ONE ones-matmul
    per (tq, head).
  * V transposes moved off the PE onto the DMA XBAR (dma_start_transpose,
    ~14ns per 32x32 tile) issued from the Scalar queue.
  * o_proj PSUM evictions moved to the otherwise-idle GpSimd (Pool) engine.
  * o_proj for block tq is issued after attention block tq+1 so its ys
    dependency (DVE reciprocal*mul) is never on the PE critical path.
  * Score tiles are computed at exact causal widths (bf16 has no N>=256
    restriction like fp32r).

Device-side layout (per core) keeps every matmul contraction on the
partition axis with no on-chip activation transposes:
  qT[h]  = (Wq_h @ x.T + bq)/sqrt(D)   [D=128 part, T free]
  kT     =  Wk_g @ x.T + bk            [128, T]
  vT     =  Wv_g @ x.T + bv            [128, T] -> XBAR-transposed to [T,128]
  RoPE on qT/kT via sign-folded sin + partition-swap DMA copies.
  ST     = k'T.T @ q'T                 [tk 128-part, tq 512-free] per head
  P      = exp(ST) (no max subtraction: logits are bounded), causal mask via
           a triangular [128,128] multiply on diagonal tiles; fully-masked
           column ranges are never computed.
  acc    = sum_tk P-tile (DVE bf16), sums = ones128.T @ acc (one matmul)
  yT     = matmul(lhsT=v[tk,128], rhs=P) accumulated over tk
  y'T    = yT * reciprocal(sums)
  out    = y'T.T @ WoT (partial; host sums over cores)
"""

import math
import numpy as np
from contextlib import ExitStack

import ml_dtypes

import concourse.bass as bass
import concourse.bacc as bacc
import concourse.tile as tile
from concourse import mybir
from concourse.bass_utils import run_bass_kernel_spmd

B, T, C = 1, 2048, 2048
H, HKV = 16, 4
D = 128
NCORES = 8
HL = H // NCORES          # query heads per core
TQ = 512                  # query tile width (one fp32 PSUM bank)
NT = T // TQ              # 4 query tiles
NK = T // D               # 16 key tiles
NCT = C // 128            # 16 contraction tiles over the model dim
F32 = mybir.dt.float32
BF16 = mybir.dt.bfloat16
Id = mybir.ActivationFunctionType.Identity
Exp = mybir.ActivationFunctionType.Exp

_CACHE: dict = {}


def _build():
    nc = bacc.Bacc(None, target_bir_lowering=False, debug=False)
    scale = 1.0 / math.sqrt(D)
    with tile.TileContext(nc) as tc, ExitStack() as ctx:
        dram = ctx.enter_context(tc.tile_pool(name="dram", bufs=1, space="DRAM"))

        def din(name, shape, dt=BF16):
            return dram.tile(shape, dt, kind="ExternalInput", name=name,
                             uniquify=False)

        xt_d = din("xt", [128, NCT * T])    # x[0].T, partition-major tiles
        wq_d = din("wq", [128, NCT * HL * D])
        wkv_d = din("wkv", [128, NCT * 2 * D])  # per c: [k 128 | v 128]
        wo_d = din("wo", [128, HL * C])
        tri_d = din("tri", [D, D])          # tri[i,j] = 1.0 if i<=j else 0.0
        one_d = din("ones", [D, D])
        bq_d = din("bq", [D, HL], F32)      # pre-scaled by 1/sqrt(D)
        bk_d = din("bk", [D, 1], F32)
        bv_d = din("bv", [D, 1], F32)
        cos_d = din("cost", [D, T])         # cos[0].T
        sin_d = din("sins", [D, T])         # sin[0].T with rows 0:64 negated
        out_d = dram.tile([T, C], BF16, kind="ExternalOutput",
                          name="out", uniquify=False)

        const = ctx.enter_context(tc.tile_pool(name="const", bufs=1))
        xt_s = const.tile([128, NCT * T], BF16, name="xt_s")
        wq_s = const.tile([128, NCT * HL * D], BF16, name="wq_s")
        wkv_s = const.tile([128, NCT * 2 * D], BF16, name="wkv_s")
        wo_s = const.tile([128, HL * C], BF16, name="wo_s")
        bq_s = const.tile([128, HL], F32, name="bq_s")
        bk_s = const.tile([128, 1], F32, name="bk_s")
        bv_s = const.tile([128, 1], F32, name="bv_s")
        cos_s = const.tile([128, T], BF16, name="cos_s")
        sin_s = const.tile([128, T], BF16, name="sin_s")
        tri_s = const.tile([128, 128], BF16, name="tri_s")
        ones_s = const.tile([128, 128], BF16, name="ones_s")

        # Weight chunks on the GpSimd SWDGE queue, x chunks on the Sync
        # queue, small consts on the Scalar queue: the first projection
        # matmul only needs the c=0 chunks, so the PE starts ~4us in
        # instead of waiting ~30us for whole-tensor loads.
        for c in range(NCT):
            nc.gpsimd.dma_start(out=wq_s[:, c * HL * D:(c + 1) * HL * D],
                                in_=wq_d[:, c * HL * D:(c + 1) * HL * D])
            nc.gpsimd.dma_start(out=wkv_s[:, c * 2 * D:(c + 1) * 2 * D],
                                in_=wkv_d[:, c * 2 * D:(c + 1) * 2 * D])
            if c == 0:  # cos/sin first needed at the first RoPE (~28us)
                nc.gpsimd.dma_start(out=cos_s[:], in_=cos_d[:])
                nc.gpsimd.dma_start(out=sin_s[:], in_=sin_d[:])
        # remaining consts ride behind the weights; all needed >25us in
        for dst, src in ((bq_s, bq_d), (bk_s, bk_d), (bv_s, bv_d),
                         (tri_s, tri_d), (ones_s, one_d)):
            nc.gpsimd.dma_start(out=dst[:], in_=src[:])
        # x streams as 8 two-chunk DMAs (8KB contiguous per partition -
        # per-row 4KB descriptors measured only ~125GB/s/queue) alternating
        # between the two HWDGE queues with nothing ahead of them.
        for g in range(NCT // 2):
            eng = nc.sync if g % 2 == 0 else nc.scalar
            eng.dma_start(out=xt_s[:, g * 2 * T:(g + 1) * 2 * T],
                          in_=xt_d[:, g * 2 * T:(g + 1) * 2 * T])

        act = ctx.enter_context(tc.tile_pool(name="act", bufs=1))
        qr = [act.tile([128, T], BF16, name=f"qr{h}_s") for h in range(HL)]
        kr_s = act.tile([128, T], BF16, name="kr_s")
        vT_s = act.tile([128, T], BF16, name="vT_s")
        v_s = act.tile([128, NK * D], BF16, name="v_s")
        ys = [act.tile([128, T], BF16, name=f"y{h}_s") for h in range(HL)]

        prepool = ctx.enter_context(tc.tile_pool(name="prepool", bufs=3))
        rpool = ctx.enter_context(tc.tile_pool(name="rpool", bufs=3))
        ppool = ctx.enter_context(tc.tile_pool(name="ppool", bufs=4))
        apool = ctx.enter_context(tc.tile_pool(name="apool", bufs=2))
        opool = ctx.enter_context(tc.tile_pool(name="opool", bufs=2))

        def rope(dst, pre, t, eng):
            """dst = pre*cos + rot_half(pre)*sin on columns [t*TQ, (t+1)*TQ).

            All operands bf16 in SBUF. The partition-swap copies ride the
            GpSimd SWDGE queue. eng picks the mul/add engine: DVE for pair 0
            (it is idle then), GpSimd for pair 1 so the attention phase's
            DVE work (tri masks, acc adds) is never queued behind RoPE.
            """
            sl = bass.ts(t, TQ)
            rot = rpool.tile([128, TQ], BF16, tag="rot")
            nc.gpsimd.dma_start(out=rot[0:64, :], in_=pre[64:128, :])
            nc.gpsimd.dma_start(out=rot[64:128, :], in_=pre[0:64, :])
            eng.tensor_mul(rot[:], rot[:], sin_s[:, sl])
            tmp = rpool.tile([128, TQ], BF16, tag="rtmp")
            eng.tensor_mul(tmp[:], pre[:], cos_s[:, sl])
            eng.tensor_add(dst, tmp[:], rot[:])

        def proj_q(tp, pa):
            """Q projection for tokens [tp*1024, (tp+1)*1024), 4 PSUM banks.

            Splitting q from k/v de-lumps the drain: this pass's evictions
            and RoPE run on ACT/DVE while the kv pass occupies the PE."""
            ps = [[pa.tile([128, TQ], F32, tag=f"pq{half}{j}",
                           name=f"pq{half}{j}")
                   for j in range(2)] for half in range(2)]
            for c in range(NCT):
                st, sp = (c == 0), (c == NCT - 1)
                xb = c * T + tp * 2 * TQ
                wqc = c * HL * D
                for j in range(2):
                    for half in range(2):
                        nc.tensor.matmul(
                            ps[half][j][:],
                            wq_s[:, wqc + j * D:wqc + (j + 1) * D],
                            xt_s[:, xb + half * TQ:xb + (half + 1) * TQ],
                            start=st, stop=sp)
            reng = nc.vector if tp == 0 else nc.gpsimd
            for half in range(2):
                t = 2 * tp + half
                for j in range(2):
                    pre = prepool.tile([128, TQ], BF16, tag=f"pre{j}")
                    nc.scalar.activation(pre[:], ps[half][j][:], Id,
                                         bias=bq_s[:, j:j + 1], scale=scale)
                    rope(qr[j][:, bass.ts(t, TQ)], pre, t, reng)

        def proj_kv(tp, pa):
            """K/V projection for the pair; the other 4 PSUM banks. V's
            XBAR transposes are issued from the Sync queue, NOT ACT
            (dma_start_transpose costs ~1.2us on the issuing engine)."""
            ps = [[pa.tile([128, TQ], F32, tag=f"pkv{half}{j}",
                           name=f"pkv{half}{j}")
                   for j in range(2)] for half in range(2)]
            for c in range(NCT):
                st, sp = (c == 0), (c == NCT - 1)
                xb = c * T + tp * 2 * TQ
                wkc = c * 2 * D
                for j in range(2):
                    for half in range(2):
                        nc.tensor.matmul(
                            ps[half][j][:],
                            wkv_s[:, wkc + j * D:wkc + (j + 1) * D],
                            xt_s[:, xb + half * TQ:xb + (half + 1) * TQ],
                            start=st, stop=sp)
            reng = nc.vector if tp == 0 else nc.gpsimd
            for half in range(2):
                t = 2 * tp + half
                sl = bass.ts(t, TQ)
                prek = prepool.tile([128, TQ], BF16, tag="prek")
                nc.scalar.activation(prek[:], ps[half][0][:], Id,
                                     bias=bk_s[:, 0:1])
                rope(kr_s[:, sl], prek, t, reng)
                # pair 1's v eviction on DVE: keeps ACT clear for the
                # attention exps that immediately follow
                if tp == 0:
                    nc.scalar.activation(vT_s[:, sl], ps[half][1][:], Id,
                                         bias=bv_s[:, 0:1])
                else:
                    nc.vector.tensor_scalar(vT_s[:, sl], ps[half][1][:],
                                            1.0, bv_s[:, 0:1],
                                            mybir.AluOpType.mult,
                                            mybir.AluOpType.add)
                for tk in range(4 * t, 4 * t + 4):
                    nc.sync.dma_start_transpose(v_s[:, bass.ts(tk, D)],
                                                vT_s[:, bass.ts(tk, D)])

        def attn_head(tq, h, pb):
            """Scores + AV for one head of query block tq."""
            ntk = 4 * tq + 4
            yp = pb.tile([128, TQ], F32, tag="yav", bufs=3)
            # denominator: two accumulator chains so the tile adds can be
            # split between DVE (fast) and GpSimd (slower but idle)
            accD = apool.tile([128, TQ], BF16, tag="accD", bufs=3)
            accG = apool.tile([128, TQ], BF16, tag="accG", bufs=3)
            nd = ng = 0
            # software-pipelined: score matmul+exp one tile ahead of the
            # consuming AV matmul so PE never waits on ACT
            pts = {}
            for tk in range(ntk + 1):
                if tk < ntk:
                    r = max(tk * D - tq * TQ, 0)  # masked col prefix
                    sp_ = pb.tile([128, TQ], F32, tag="s", bufs=2)
                    nc.tensor.matmul(
                        sp_[:, r:], kr_s[:, bass.ts(tk, D)],
                        qr[h][:, tq * TQ + r:(tq + 1) * TQ],
                        start=True, stop=True)
                    pt = ppool.tile([128, TQ], BF16, tag="p")
                    nc.scalar.activation(pt[:, r:], sp_[:, r:], Exp)
                    if tk * D >= tq * TQ:  # diagonal: causal mask
                        nc.vector.tensor_mul(pt[:, r:r + D],
                                             pt[:, r:r + D], tri_s[:])
                    # GpSimd only helps for tq>=2: earlier it is still
                    # working through pair 1's RoPE backlog
                    if tq < 2 or tk % 3 == 0:
                        eng, acc = nc.vector, accD
                        nd += 1
                        first = nd == 1
                    else:
                        eng, acc = nc.gpsimd, accG
                        ng += 1
                        first = ng == 1
                    if first and r > 0:
                        eng.memset(acc[:, 0:r], 0.0)
                    if first:
                        eng.tensor_copy(acc[:, r:], pt[:, r:])
                    else:
                        eng.tensor_add(acc[:, r:], acc[:, r:], pt[:, r:])
                    pts[tk] = (pt, r)
                if tk >= 1:
                    pt, r = pts.pop(tk - 1)
                    nc.tensor.matmul(yp[:, r:],
                                     v_s[:, bass.ts(tk - 1, D)],
                                     pt[:, r:], start=(tk - 1 == 0),
                                     stop=(tk - 1 == ntk - 1))
            return (yp, accD, accG, ng)

        def attn_fin(tq, h, fin, pb):
            """Denominator matmul + y normalization for (tq, h). Emitted
            1-2 head-blocks behind attn_head so the acc chains and the
            recip/ymul are never on the PE critical path."""
            yp, accD, accG, ng = fin
            sump = pb.tile([128, TQ], F32, tag="sum", bufs=1)
            nc.tensor.matmul(sump[:], ones_s[:], accD[:],
                             start=True, stop=(ng == 0))
            if ng:
                nc.tensor.matmul(sump[:], ones_s[:], accG[:],
                                 start=False, stop=True)
            rec = rpool.tile([128, TQ], F32, tag="rec")
            nc.vector.reciprocal_approx_fast(rec[:], sump[:])
            nc.vector.tensor_mul(ys[h][:, bass.ts(tq, TQ)], yp[:], rec[:])

        def oproj(tq, pb):
            """o_proj partial for the 4 row-tiles of query block tq.
            Evictions ride the idle GpSimd; one wide DMA per row-tile."""
            for tt in range(4):
                t = 4 * tq + tt
                wide = opool.tile([128, C], BF16, tag="oev")
                for n in range(NT):
                    op_ = pb.tile([128, TQ], F32, tag="o", bufs=2)
                    for h in range(HL):
                        nc.tensor.matmul(
                            op_[:], ys[h][:, bass.ts(t, D)],
                            wo_s[:, h * C + n * TQ:h * C + (n + 1) * TQ],
                            start=(h == 0), stop=(h == HL - 1))
                    # evictions split ACT/DVE 50/50 to balance both
                    # engines against exp/adds; GpSimd cannot read PSUM
                    if (t * NT + n) % 2 == 0:
                        nc.scalar.activation(wide[:, bass.ts(n, TQ)],
                                             op_[:], Id)
                    else:
                        nc.vector.tensor_copy(wide[:, bass.ts(n, TQ)],
                                              op_[:])
                    if n == 1:  # first half done -> overlap DMA with n=2,3
                        nc.sync.dma_start(
                            out=out_d[bass.ts(t, D), 0:2 * TQ],
                            in_=wide[:, 0:2 * TQ])
                nc.sync.dma_start(out=out_d[bass.ts(t, D), 2 * TQ:],
                                  in_=wide[:, 2 * TQ:])

        with tc.tile_pool(name="pa_psum", bufs=1, space="PSUM") as pa:
            proj_q(0, pa)
            proj_kv(0, pa)
            nc.gpsimd.dma_start(out=wo_s[:], in_=wo_d[:])
            proj_q(1, pa)
            proj_kv(1, pa)
        # Pipelined schedule: fin(tq,h) trails its attn_head by 1-2 head
        # blocks (acc chains/recip/ymul finish off the critical path), and
        # oproj(tq) trails fin(tq,h1). PSUM: s*2 + yav*3 + sum*1 + o*2 = 8.
        with tc.tile_pool(name="pb_psum", bufs=1, space="PSUM") as pb:
            f = {}
            f[0, 0] = attn_head(0, 0, pb)
            f[0, 1] = attn_head(0, 1, pb)
            attn_fin(0, 0, f[0, 0], pb)
            f[1, 0] = attn_head(1, 0, pb)
            attn_fin(0, 1, f[0, 1], pb)
            f[1, 1] = attn_head(1, 1, pb)
            attn_fin(1, 0, f[1, 0], pb)
            oproj(0, pb)
            f[2, 0] = attn_head(2, 0, pb)
            attn_fin(1, 1, f[1, 1], pb)
            f[2, 1] = attn_head(2, 1, pb)
            attn_fin(2, 0, f[2, 0], pb)
            oproj(1, pb)
            f[3, 0] = attn_head(3, 0, pb)
            attn_fin(2, 1, f[2, 1], pb)
            f[3, 1] = attn_head(3, 1, pb)
            attn_fin(3, 0, f[3, 0], pb)
            oproj(2, pb)
            attn_fin(3, 1, f[3, 1], pb)
            oproj(3, pb)
    nc.compile()
    return nc


def _get_nc():
    if "nc" not in _CACHE:
        _CACHE["nc"] = _build()
    return _CACHE["nc"]


def _prep_inputs(x, cos, sin, Wq, bq, Wk, bk, Wv, bv, Wo):
    f = np.float32
    bf = ml_dtypes.bfloat16
    xT = np.asarray(x[0].T, dtype=np.float32)
    # partition-major tiling to match the SBUF layout: one contiguous
    # [128, k*2048] slab per contraction chunk
    xT = np.ascontiguousarray(
        xT.reshape(NCT, 128, T).transpose(1, 0, 2).reshape(128, -1), dtype=bf)
    cosT = np.ascontiguousarray(cos[0].T, dtype=bf)
    sinT = np.asarray(sin[0].T, dtype=f)
    sins = np.concatenate([-sinT[:64], sinT[64:]], axis=0)
    sins = np.ascontiguousarray(sins, dtype=bf)
    idx = np.arange(D)
    tri = (idx[:, None] <= idx[None, :]).astype(bf)
    ones = np.ones((D, D), dtype=bf)
    scale = np.float32(1.0 / math.sqrt(D))
    in_maps = []

    def ptile(a):
        """[K*128, N] -> partition-major [128, K*N] matching the SBUF tiles."""
        k = a.shape[0] // 128
        return a.reshape(k, 128, a.shape[1]).transpose(1, 0, 2).reshape(128, -1)

    for m in range(NCORES):
        g = m // 2
        wq_m = np.ascontiguousarray(
            ptile(Wq[m * 256:(m + 1) * 256, :].T.astype(f)), dtype=bf)
        wk_m = ptile(Wk[g * 128:(g + 1) * 128, :].T.astype(f))
        wv_m = ptile(Wv[g * 128:(g + 1) * 128, :].T.astype(f))
        # interleave [k | v] per contraction chunk
        wkv_m = np.empty((128, NCT * 2 * D), dtype=f)
        for c in range(NCT):
            wkv_m[:, c * 2 * D:c * 2 * D + D] = wk_m[:, c * D:(c + 1) * D]
            wkv_m[:, c * 2 * D + D:(c + 1) * 2 * D] = wv_m[:, c * D:(c + 1) * D]
        wkv_m = np.ascontiguousarray(wkv_m, dtype=bf)
        wo_m = np.ascontiguousarray(
            ptile(Wo[:, m * 256:(m + 1) * 256].T.astype(f)), dtype=bf)
        bq_m = np.ascontiguousarray(
            (bq[m * 256:(m + 1) * 256] * scale).reshape(HL, D).T, dtype=f)
        bk_m = np.ascontiguousarray(bk[g * 128:(g + 1) * 128].reshape(D, 1),
                                    dtype=f)
        bv_m = np.ascontiguousarray(bv[g * 128:(g + 1) * 128].reshape(D, 1),
                                    dtype=f)
        in_maps.append({
            "xt": xT, "wq": wq_m, "wkv": wkv_m, "wo": wo_m,
            "bq": bq_m, "bk": bk_m, "bv": bv_m,
            "cost": cosT, "sins": sins, "tri": tri, "ones": ones,
        })
    return in_maps


def kernel(x, cos, sin, Wq, bq, Wk, bk, Wv, bv, Wo, _trace=False):
    x, cos, sin = np.asarray(x), np.asarray(cos), np.asarray(sin)
    Wq, bq = np.asarray(Wq), np.asarray(bq)
    Wk, bk = np.asarray(Wk), np.asarray(bk)
    Wv, bv = np.asarray(Wv), np.asarray(bv)
    Wo = np.asarray(Wo)
    nc = _get_nc()
    in_maps = _prep_inputs(x, cos, sin, Wq, bq, Wk, bk, Wv, bv, Wo)
    res = run_bass_kernel_spmd(nc, in_maps, core_ids=list(range(NCORES)),
                               trace=_trace)
    out = res.results[0]["out"].astype(np.float64)
    for m in range(1, NCORES):
        out += res.results[m]["out"]
    out = out.astype(np.float32).reshape(B, T, C)
    if _trace:
        _CACHE["last_result"] = res
    return out


# revision 32
# speedup vs baseline: 1.0327x; 1.0327x over previous
"""Trainium2 Bass kernel: causal self-attention (GQA + RoPE) for
B=1, T=2048, C=2048, H=16 query heads, HKV=4 KV heads, D=128.

Sharding: tensor-parallel over heads across 8 NeuronCores. Core m computes
query heads {2m, 2m+1} and the single KV head (m//2) those heads attend to,
plus the o_proj partial product for its 256 input columns. The host sums the
8 partial outputs (the TP all-reduce).

v2 (vs the fp32r baseline at ~219us):
  * All matmul operands are bf16 (tolerance is 2e-2; baseline err 1.9e-3).
    Halves HBM traffic and SBUF footprint, and DVE elementwise ops hit the
    2x/4x 16-bit perf modes.
  * The 45us DMA lead-in is gone: weights stream in [128,256] c-chunks on
    the GpSimd queue interleaved with [128,2048] x c-chunks on the Sync
    queue, so the first projection matmul only waits for chunk 0 (~4us).
  * The 80 ones-matmul softmax-denominator reductions (~19us of PE) are
    replaced by bf16 DVE accumulations of the exp tiles plus ONE ones-matmul
    per (tq, head).
  * V transposes moved off the PE onto the DMA XBAR (dma_start_transpose,
    ~14ns per 32x32 tile) issued from the Scalar queue.
  * o_proj PSUM evictions moved to the otherwise-idle GpSimd (Pool) engine.
  * o_proj for block tq is issued after attention block tq+1 so its ys
    dependency (DVE reciprocal*mul) is never on the PE critical path.
  * Score tiles are computed at exact causal widths (bf16 has no N>=256
    restriction like fp32r).

Device-side layout (per core) keeps every matmul contraction on the
partition axis with no on-chip activation transposes:
  qT[h]  = (Wq_h @ x.T + bq)/sqrt(D)   [D=128 part, T free]
  kT     =  Wk_g @ x.T + bk            [128, T]
  vT     =  Wv_g @ x.T + bv            [128, T] -> XBAR-transposed to [T,128]
  RoPE on qT/kT via sign-folded sin + partition-swap DMA copies.
  ST     = k'T.T @ q'T                 [tk 128-part, tq 512-free] per head
  P      = exp(ST) (no max subtraction: logits are bounded), causal mask via
           a triangular [128,128] multiply on diagonal tiles; fully-masked
           column ranges are never computed.
  acc    = sum_tk P-tile (DVE bf16), sums = ones128.T @ acc (one matmul)
  yT     = matmul(lhsT=v[tk,128], rhs=P) accumulated over tk
  y'T    = yT * reciprocal(sums)
  out    = y'T.T @ WoT (partial; host sums over cores)
"""

import math
import numpy as np
from contextlib import ExitStack

import ml_dtypes

import concourse.bass as bass
import concourse.bacc as bacc
import concourse.tile as tile
from concourse import mybir
from concourse.bass_utils import run_bass_kernel_spmd

B, T, C = 1, 2048, 2048
H, HKV = 16, 4
D = 128
NCORES = 8
HL = H // NCORES          # query heads per core
TQ = 512                  # query tile width (one fp32 PSUM bank)
NT = T // TQ              # 4 query tiles
NK = T // D               # 16 key tiles
NCT = C // 128            # 16 contraction tiles over the model dim
F32 = mybir.dt.float32
BF16 = mybir.dt.bfloat16
Id = mybir.ActivationFunctionType.Identity
Exp = mybir.ActivationFunctionType.Exp

_CACHE: dict = {}


def _build():
    nc = bacc.Bacc(None, target_bir_lowering=False, debug=False)
    scale = 1.0 / math.sqrt(D)
    with tile.TileContext(nc) as tc, ExitStack() as ctx:
        dram = ctx.enter_context(tc.tile_pool(name="dram", bufs=1, space="DRAM"))

        def din(name, shape, dt=BF16):
            return dram.tile(shape, dt, kind="ExternalInput", name=name,
                             uniquify=False)

        xt_d = din("xt", [128, NCT * T])    # x[0].T, partition-major tiles
        wq_d = din("wq", [128, NCT * HL * D])
        wkv_d = din("wkv", [128, NCT * 2 * D])  # per c: [k 128 | v 128]
        wo_d = din("wo", [128, HL * C])
        tri_d = din("tri", [D, D])          # tri[i,j] = 1.0 if i<=j else 0.0
        one_d = din("ones", [D, D])
        bq_d = din("bq", [D, HL], F32)      # pre-scaled by 1/sqrt(D)
        bk_d = din("bk", [D, 1], F32)
        bv_d = din("bv", [D, 1], F32)
        cos_d = din("cost", [D, T])         # cos[0].T
        sin_d = din("sins", [D, T])         # sin[0].T with rows 0:64 negated
        out_d = dram.tile([T, C], BF16, kind="ExternalOutput",
                          name="out", uniquify=False)

        const = ctx.enter_context(tc.tile_pool(name="const", bufs=1))
        xt_s = const.tile([128, NCT * T], BF16, name="xt_s")
        wq_s = const.tile([128, NCT * HL * D], BF16, name="wq_s")
        wkv_s = const.tile([128, NCT * 2 * D], BF16, name="wkv_s")
        wo_s = const.tile([128, HL * C], BF16, name="wo_s")
        bq_s = const.tile([128, HL], F32, name="bq_s")
        bk_s = const.tile([128, 1], F32, name="bk_s")
        bv_s = const.tile([128, 1], F32, name="bv_s")
        cos_s = const.tile([128, T], BF16, name="cos_s")
        sin_s = const.tile([128, T], BF16, name="sin_s")
        tri_s = const.tile([128, 128], BF16, name="tri_s")
        ones_s = const.tile([128, 128], BF16, name="ones_s")

        # Weight chunks on the GpSimd SWDGE queue, x chunks on the Sync
        # queue, small consts on the Scalar queue: the first projection
        # matmul only needs the c=0 chunks, so the PE starts ~4us in
        # instead of waiting ~30us for whole-tensor loads.
        for c in range(NCT):
            nc.gpsimd.dma_start(out=wq_s[:, c * HL * D:(c + 1) * HL * D],
                                in_=wq_d[:, c * HL * D:(c + 1) * HL * D])
            nc.gpsimd.dma_start(out=wkv_s[:, c * 2 * D:(c + 1) * 2 * D],
                                in_=wkv_d[:, c * 2 * D:(c + 1) * 2 * D])
        # consts ride behind the weights; all needed >25us in
        for dst, src in ((bq_s, bq_d), (bk_s, bk_d), (bv_s, bv_d),
                         (cos_s, cos_d), (sin_s, sin_d), (tri_s, tri_d),
                         (ones_s, one_d)):
            nc.gpsimd.dma_start(out=dst[:], in_=src[:])
        # x streams per-chunk on the two HWDGE queues with nothing ahead of
        # them; the 8MB stream is HBM-bound (~23us) either way, but small
        # chunks give the earliest c=0 arrival and per-chunk dependencies.
        for c in range(NCT):
            eng = nc.sync if c % 2 == 0 else nc.scalar
            eng.dma_start(out=xt_s[:, c * T:(c + 1) * T],
                          in_=xt_d[:, c * T:(c + 1) * T])

        act = ctx.enter_context(tc.tile_pool(name="act", bufs=1))
        qr = [act.tile([128, T], BF16, name=f"qr{h}_s") for h in range(HL)]
        kr_s = act.tile([128, T], BF16, name="kr_s")
        vT_s = act.tile([128, T], BF16, name="vT_s")
        v_s = act.tile([128, NK * D], BF16, name="v_s")
        ys = [act.tile([128, T], BF16, name=f"y{h}_s") for h in range(HL)]

        prepool = ctx.enter_context(tc.tile_pool(name="prepool", bufs=3))
        rpool = ctx.enter_context(tc.tile_pool(name="rpool", bufs=3))
        ppool = ctx.enter_context(tc.tile_pool(name="ppool", bufs=4))
        apool = ctx.enter_context(tc.tile_pool(name="apool", bufs=2))
        opool = ctx.enter_context(tc.tile_pool(name="opool", bufs=2))

        def rope(dst, pre, t, eng):
            """dst = pre*cos + rot_half(pre)*sin on columns [t*TQ, (t+1)*TQ).

            All operands bf16 in SBUF. The partition-swap copies ride the
            GpSimd SWDGE queue. eng picks the mul/add engine: DVE for pair 0
            (it is idle then), GpSimd for pair 1 so the attention phase's
            DVE work (tri masks, acc adds) is never queued behind RoPE.
            """
            sl = bass.ts(t, TQ)
            rot = rpool.tile([128, TQ], BF16, tag="rot")
            nc.gpsimd.dma_start(out=rot[0:64, :], in_=pre[64:128, :])
            nc.gpsimd.dma_start(out=rot[64:128, :], in_=pre[0:64, :])
            eng.tensor_mul(rot[:], rot[:], sin_s[:, sl])
            tmp = rpool.tile([128, TQ], BF16, tag="rtmp")
            eng.tensor_mul(tmp[:], pre[:], cos_s[:, sl])
            eng.tensor_add(dst, tmp[:], rot[:])

        def proj_pair(tp, pa):
            """Combined QKV projection for pair tp (8 PSUM banks). Used for
            pair 0: its 1.84us/chunk consumption matches the ~1.4-2us/chunk
            HBM supply rate of the x stream (a split q-pass would consume
            2x faster than HBM can feed it)."""
            ps = []  # [half][q0, q1, k, v]
            for half in range(2):
                ps.append([pa.tile([128, TQ], F32, tag=f"pp{half}{j}",
                                   name=f"pp{half}{j}")
                           for j in range(4)])
            for c in range(NCT):
                st, sp = (c == 0), (c == NCT - 1)
                xb = c * T + tp * 2 * TQ
                wqc, wkc = c * HL * D, c * 2 * D
                for j, wsl in ((0, wq_s[:, wqc:wqc + D]),
                               (1, wq_s[:, wqc + D:wqc + 2 * D]),
                               (2, wkv_s[:, wkc:wkc + D]),
                               (3, wkv_s[:, wkc + D:wkc + 2 * D])):
                    for half in range(2):
                        nc.tensor.matmul(
                            ps[half][j][:], wsl,
                            xt_s[:, xb + half * TQ:xb + (half + 1) * TQ],
                            start=st, stop=sp)
            reng = nc.vector
            for half in range(2):
                t = 2 * tp + half
                sl = bass.ts(t, TQ)
                prek = prepool.tile([128, TQ], BF16, tag="prek")
                nc.scalar.activation(prek[:], ps[half][2][:], Id,
                                     bias=bk_s[:, 0:1])
                rope(kr_s[:, sl], prek, t, reng)
                for j in range(2):
                    pre = prepool.tile([128, TQ], BF16, tag=f"pre{j}")
                    nc.scalar.activation(pre[:], ps[half][j][:], Id,
                                         bias=bq_s[:, j:j + 1], scale=scale)
                    rope(qr[j][:, bass.ts(t, TQ)], pre, t, reng)
                nc.scalar.activation(vT_s[:, sl], ps[half][3][:], Id,
                                     bias=bv_s[:, 0:1])
                for tk in range(4 * t, 4 * t + 4):
                    nc.sync.dma_start_transpose(v_s[:, bass.ts(tk, D)],
                                                vT_s[:, bass.ts(tk, D)])

        def proj_q(tp, pa):
            """Q projection only (4 PSUM banks, shared tags with
            proj_pair). Used for pair 1 when x is already resident:
            splitting q from k/v de-lumps the drain so this pass's
            evictions and RoPE run on ACT/GpSimd while the kv pass
            occupies the PE."""
            ps = [[pa.tile([128, TQ], F32, tag=f"pp{half}{j}",
                           name=f"pq{half}{j}")
                   for j in range(2)] for half in range(2)]
            for c in range(NCT):
                st, sp = (c == 0), (c == NCT - 1)
                xb = c * T + tp * 2 * TQ
                wqc = c * HL * D
                for j in range(2):
                    for half in range(2):
                        nc.tensor.matmul(
                            ps[half][j][:],
                            wq_s[:, wqc + j * D:wqc + (j + 1) * D],
                            xt_s[:, xb + half * TQ:xb + (half + 1) * TQ],
                            start=st, stop=sp)
            reng = nc.gpsimd
            for half in range(2):
                t = 2 * tp + half
                for j in range(2):
                    pre = prepool.tile([128, TQ], BF16, tag=f"pre{j}")
                    nc.scalar.activation(pre[:], ps[half][j][:], Id,
                                         bias=bq_s[:, j:j + 1], scale=scale)
                    rope(qr[j][:, bass.ts(t, TQ)], pre, t, reng)

        def proj_kv(tp, pa):
            """K/V projection (the other 4 banks, shared tags). V's XBAR
            transposes are issued from the Sync queue, NOT ACT
            (dma_start_transpose costs ~1.2us on the issuing engine)."""
            ps = [[pa.tile([128, TQ], F32, tag=f"pp{half}{j + 2}",
                           name=f"pkv{half}{j}")
                   for j in range(2)] for half in range(2)]
            for c in range(NCT):
                st, sp = (c == 0), (c == NCT - 1)
                xb = c * T + tp * 2 * TQ
                wkc = c * 2 * D
                for j in range(2):
                    for half in range(2):
                        nc.tensor.matmul(
                            ps[half][j][:],
                            wkv_s[:, wkc + j * D:wkc + (j + 1) * D],
                            xt_s[:, xb + half * TQ:xb + (half + 1) * TQ],
                            start=st, stop=sp)
            for half in range(2):
                t = 2 * tp + half
                sl = bass.ts(t, TQ)
                prek = prepool.tile([128, TQ], BF16, tag="prek")
                nc.scalar.activation(prek[:], ps[half][0][:], Id,
                                     bias=bk_s[:, 0:1])
                rope(kr_s[:, sl], prek, t, nc.gpsimd)
                # v eviction on DVE: keeps ACT clear for the attention
                # exps that immediately follow
                nc.vector.tensor_scalar(vT_s[:, sl], ps[half][1][:],
                                        1.0, bv_s[:, 0:1],
                                        mybir.AluOpType.mult,
                                        mybir.AluOpType.add)
                for tk in range(4 * t, 4 * t + 4):
                    nc.sync.dma_start_transpose(v_s[:, bass.ts(tk, D)],
                                                vT_s[:, bass.ts(tk, D)])

        def attn_head(tq, h, pb):
            """Scores + AV for one head of query block tq."""
            ntk = 4 * tq + 4
            yp = pb.tile([128, TQ], F32, tag="yav", bufs=3)
            # denominator: two accumulator chains so the tile adds can be
            # split between DVE (fast) and GpSimd (slower but idle)
            accD = apool.tile([128, TQ], BF16, tag="accD", bufs=3)
            accG = apool.tile([128, TQ], BF16, tag="accG", bufs=3)
            nd = ng = 0
            # software-pipelined: score matmul+exp one tile ahead of the
            # consuming AV matmul so PE never waits on ACT
            pts = {}
            for tk in range(ntk + 1):
                if tk < ntk:
                    r = max(tk * D - tq * TQ, 0)  # masked col prefix
                    sp_ = pb.tile([128, TQ], F32, tag="s", bufs=2)
                    nc.tensor.matmul(
                        sp_[:, r:], kr_s[:, bass.ts(tk, D)],
                        qr[h][:, tq * TQ + r:(tq + 1) * TQ],
                        start=True, stop=True)
                    pt = ppool.tile([128, TQ], BF16, tag="p")
                    nc.scalar.activation(pt[:, r:], sp_[:, r:], Exp)
                    if tk * D >= tq * TQ:  # diagonal: causal mask
                        nc.vector.tensor_mul(pt[:, r:r + D],
                                             pt[:, r:r + D], tri_s[:])
                    # GpSimd only helps for tq>=2: earlier it is still
                    # working through pair 1's RoPE backlog
                    if tq < 2 or tk % 3 == 0:
                        eng, acc = nc.vector, accD
                        nd += 1
                        first = nd == 1
                    else:
                        eng, acc = nc.gpsimd, accG
                        ng += 1
                        first = ng == 1
                    if first and r > 0:
                        eng.memset(acc[:, 0:r], 0.0)
                    if first:
                        eng.tensor_copy(acc[:, r:], pt[:, r:])
                    else:
                        eng.tensor_add(acc[:, r:], acc[:, r:], pt[:, r:])
                    pts[tk] = (pt, r)
                if tk >= 1:
                    pt, r = pts.pop(tk - 1)
                    nc.tensor.matmul(yp[:, r:],
                                     v_s[:, bass.ts(tk - 1, D)],
                                     pt[:, r:], start=(tk - 1 == 0),
                                     stop=(tk - 1 == ntk - 1))
            return (yp, accD, accG, ng)

        def attn_fin(tq, h, fin, pb):
            """Denominator matmul + y normalization for (tq, h). Emitted
            1-2 head-blocks behind attn_head so the acc chains and the
            recip/ymul are never on the PE critical path."""
            yp, accD, accG, ng = fin
            sump = pb.tile([128, TQ], F32, tag="sum", bufs=1)
            nc.tensor.matmul(sump[:], ones_s[:], accD[:],
                             start=True, stop=(ng == 0))
            if ng:
                nc.tensor.matmul(sump[:], ones_s[:], accG[:],
                                 start=False, stop=True)
            rec = rpool.tile([128, TQ], F32, tag="rec")
            nc.vector.reciprocal_approx_fast(rec[:], sump[:])
            nc.vector.tensor_mul(ys[h][:, bass.ts(tq, TQ)], yp[:], rec[:])

        def oproj(tq, pb):
            """o_proj partial for the 4 row-tiles of query block tq.
            Evictions ride the idle GpSimd; one wide DMA per row-tile."""
            for tt in range(4):
                t = 4 * tq + tt
                wide = opool.tile([128, C], BF16, tag="oev")
                for n in range(NT):
                    op_ = pb.tile([128, TQ], F32, tag="o", bufs=2)
                    for h in range(HL):
                        nc.tensor.matmul(
                            op_[:], ys[h][:, bass.ts(t, D)],
                            wo_s[:, h * C + n * TQ:h * C + (n + 1) * TQ],
                            start=(h == 0), stop=(h == HL - 1))
                    # evictions split ACT/DVE 50/50 to balance both
                    # engines against exp/adds; GpSimd cannot read PSUM
                    if (t * NT + n) % 2 == 0:
                        nc.scalar.activation(wide[:, bass.ts(n, TQ)],
                                             op_[:], Id)
                    else:
                        nc.vector.tensor_copy(wide[:, bass.ts(n, TQ)],
                                              op_[:])
                    if n == 1:  # first half done -> overlap DMA with n=2,3
                        nc.sync.dma_start(
                            out=out_d[bass.ts(t, D), 0:2 * TQ],
                            in_=wide[:, 0:2 * TQ])
                nc.sync.dma_start(out=out_d[bass.ts(t, D), 2 * TQ:],
                                  in_=wide[:, 2 * TQ:])

        with tc.tile_pool(name="pa_psum", bufs=1, space="PSUM") as pa:
            proj_pair(0, pa)
            nc.gpsimd.dma_start(out=wo_s[:], in_=wo_d[:])
            proj_q(1, pa)
            proj_kv(1, pa)
        # Pipelined schedule: fin(tq,h) trails its attn_head by 1-2 head
        # blocks (acc chains/recip/ymul finish off the critical path), and
        # oproj(tq) trails fin(tq,h1). PSUM: s*2 + yav*3 + sum*1 + o*2 = 8.
        with tc.tile_pool(name="pb_psum", bufs=1, space="PSUM") as pb:
            f = {}
            f[0, 0] = attn_head(0, 0, pb)
            f[0, 1] = attn_head(0, 1, pb)
            attn_fin(0, 0, f[0, 0], pb)
            f[1, 0] = attn_head(1, 0, pb)
            attn_fin(0, 1, f[0, 1], pb)
            f[1, 1] = attn_head(1, 1, pb)
            attn_fin(1, 0, f[1, 0], pb)
            oproj(0, pb)
            f[2, 0] = attn_head(2, 0, pb)
            attn_fin(1, 1, f[1, 1], pb)
            f[2, 1] = attn_head(2, 1, pb)
            attn_fin(2, 0, f[2, 0], pb)
            oproj(1, pb)
            f[3, 0] = attn_head(3, 0, pb)
            attn_fin(2, 1, f[2, 1], pb)
            f[3, 1] = attn_head(3, 1, pb)
            attn_fin(3, 0, f[3, 0], pb)
            oproj(2, pb)
            attn_fin(3, 1, f[3, 1], pb)
            oproj(3, pb)
    nc.compile()
    return nc


def _get_nc():
    if "nc" not in _CACHE:
        _CACHE["nc"] = _build()
    return _CACHE["nc"]


def _prep_inputs(x, cos, sin, Wq, bq, Wk, bk, Wv, bv, Wo):
    f = np.float32
    bf = ml_dtypes.bfloat16
    xT = np.asarray(x[0].T, dtype=np.float32)
    # partition-major tiling to match the SBUF layout: one contiguous
    # [128, k*2048] slab per contraction chunk
    xT = np.ascontiguousarray(
        xT.reshape(NCT, 128, T).transpose(1, 0, 2).reshape(128, -1), dtype=bf)
    cosT = np.ascontiguousarray(cos[0].T, dtype=bf)
    sinT = np.asarray(sin[0].T, dtype=f)
    sins = np.concatenate([-sinT[:64], sinT[64:]], axis=0)
    sins = np.ascontiguousarray(sins, dtype=bf)
    idx = np.arange(D)
    tri = (idx[:, None] <= idx[None, :]).astype(bf)
    ones = np.ones((D, D), dtype=bf)
    scale = np.float32(1.0 / math.sqrt(D))
    in_maps = []

    def ptile(a):
        """[K*128, N] -> partition-major [128, K*N] matching the SBUF tiles."""
        k = a.shape[0] // 128
        return a.reshape(k, 128, a.shape[1]).transpose(1, 0, 2).reshape(128, -1)

    for m in range(NCORES):
        g = m // 2
        wq_m = np.ascontiguousarray(
            ptile(Wq[m * 256:(m + 1) * 256, :].T.astype(f)), dtype=bf)
        wk_m = ptile(Wk[g * 128:(g + 1) * 128, :].T.astype(f))
        wv_m = ptile(Wv[g * 128:(g + 1) * 128, :].T.astype(f))
        # interleave [k | v] per contraction chunk
        wkv_m = np.empty((128, NCT * 2 * D), dtype=f)
        for c in range(NCT):
            wkv_m[:, c * 2 * D:c * 2 * D + D] = wk_m[:, c * D:(c + 1) * D]
            wkv_m[:, c * 2 * D + D:(c + 1) * 2 * D] = wv_m[:, c * D:(c + 1) * D]
        wkv_m = np.ascontiguousarray(wkv_m, dtype=bf)
        wo_m = np.ascontiguousarray(
            ptile(Wo[:, m * 256:(m + 1) * 256].T.astype(f)), dtype=bf)
        bq_m = np.ascontiguousarray(
            (bq[m * 256:(m + 1) * 256] * scale).reshape(HL, D).T, dtype=f)
        bk_m = np.ascontiguousarray(bk[g * 128:(g + 1) * 128].reshape(D, 1),
                                    dtype=f)
        bv_m = np.ascontiguousarray(bv[g * 128:(g + 1) * 128].reshape(D, 1),
                                    dtype=f)
        in_maps.append({
            "xt": xT, "wq": wq_m, "wkv": wkv_m, "wo": wo_m,
            "bq": bq_m, "bk": bk_m, "bv": bv_m,
            "cost": cosT, "sins": sins, "tri": tri, "ones": ones,
        })
    return in_maps


def kernel(x, cos, sin, Wq, bq, Wk, bk, Wv, bv, Wo, _trace=False):
    x, cos, sin = np.asarray(x), np.asarray(cos), np.asarray(sin)
    Wq, bq = np.asarray(Wq), np.asarray(bq)
    Wk, bk = np.asarray(Wk), np.asarray(bk)
    Wv, bv = np.asarray(Wv), np.asarray(bv)
    Wo = np.asarray(Wo)
    nc = _get_nc()
    in_maps = _prep_inputs(x, cos, sin, Wq, bq, Wk, bk, Wv, bv, Wo)
    res = run_bass_kernel_spmd(nc, in_maps, core_ids=list(range(NCORES)),
                               trace=_trace)
    out = res.results[0]["out"].astype(np.float64)
    for m in range(1, NCORES):
        out += res.results[m]["out"]
    out = out.astype(np.float32).reshape(B, T, C)
    if _trace:
        _CACHE["last_result"] = res
    return out


# revision 39
# speedup vs baseline: 1.0687x; 1.0348x over previous
"""Trainium2 Bass kernel: causal self-attention (GQA + RoPE) for
B=1, T=2048, C=2048, H=16 query heads, HKV=4 KV heads, D=128.

Sharding: tensor-parallel over heads across 8 NeuronCores. Core m computes
query heads {2m, 2m+1} and the single KV head (m//2) those heads attend to,
plus the o_proj partial product for its 256 input columns. The host sums the
8 partial outputs (the TP all-reduce).

v2 (vs the fp32r baseline at ~219us):
  * All matmul operands are bf16 (tolerance is 2e-2; baseline err 1.9e-3).
    Halves HBM traffic and SBUF footprint, and DVE elementwise ops hit the
    2x/4x 16-bit perf modes.
  * The 45us DMA lead-in is gone: weights stream in [128,256] c-chunks on
    the GpSimd queue interleaved with [128,2048] x c-chunks on the Sync
    queue, so the first projection matmul only waits for chunk 0 (~4us).
  * The 80 ones-matmul softmax-denominator reductions (~19us of PE) are
    replaced by bf16 DVE accumulations of the exp tiles plus ONE ones-matmul
    per (tq, head).
  * V transposes moved off the PE onto the DMA XBAR (dma_start_transpose,
    ~14ns per 32x32 tile) issued from the Scalar queue.
  * o_proj PSUM evictions moved to the otherwise-idle GpSimd (Pool) engine.
  * o_proj for block tq is issued after attention block tq+1 so its ys
    dependency (DVE reciprocal*mul) is never on the PE critical path.
  * Score tiles are computed at exact causal widths (bf16 has no N>=256
    restriction like fp32r).

Device-side layout (per core) keeps every matmul contraction on the
partition axis with no on-chip activation transposes:
  qT[h]  = (Wq_h @ x.T + bq)/sqrt(D)   [D=128 part, T free]
  kT     =  Wk_g @ x.T + bk            [128, T]
  vT     =  Wv_g @ x.T + bv            [128, T] -> XBAR-transposed to [T,128]
  RoPE on qT/kT via sign-folded sin + partition-swap DMA copies.
  ST     = k'T.T @ q'T                 [tk 128-part, tq 512-free] per head
  P      = exp(ST) (no max subtraction: logits are bounded), causal mask via
           a triangular [128,128] multiply on diagonal tiles; fully-masked
           column ranges are never computed.
  acc    = sum_tk P-tile (DVE bf16), sums = ones128.T @ acc (one matmul)
  yT     = matmul(lhsT=v[tk,128], rhs=P) accumulated over tk
  y'T    = yT * reciprocal(sums)
  out    = y'T.T @ WoT (partial; host sums over cores)
"""

import math
import numpy as np
from contextlib import ExitStack

import ml_dtypes

import concourse.bass as bass
import concourse.bacc as bacc
import concourse.tile as tile
from concourse import mybir
from concourse.bass_utils import run_bass_kernel_spmd

B, T, C = 1, 2048, 2048
H, HKV = 16, 4
D = 128
NCORES = 8
HL = H // NCORES          # query heads per core
TQ = 512                  # query tile width (one fp32 PSUM bank)
NT = T // TQ              # 4 query tiles
NK = T // D               # 16 key tiles
NCT = C // 128            # 16 contraction tiles over the model dim
F32 = mybir.dt.float32
BF16 = mybir.dt.bfloat16
Id = mybir.ActivationFunctionType.Identity
Exp = mybir.ActivationFunctionType.Exp

_CACHE: dict = {}


def _build():
    nc = bacc.Bacc(None, target_bir_lowering=False, debug=False)
    scale = 1.0 / math.sqrt(D)
    with tile.TileContext(nc) as tc, ExitStack() as ctx:
        dram = ctx.enter_context(tc.tile_pool(name="dram", bufs=1, space="DRAM"))

        def din(name, shape, dt=BF16):
            return dram.tile(shape, dt, kind="ExternalInput", name=name,
                             uniquify=False)

        xt_d = din("xt", [128, NCT * T])    # x[0].T, partition-major tiles
        wq_d = din("wq", [128, NCT * HL * D])
        wkv_d = din("wkv", [128, NCT * 2 * D])  # per c: [k 128 | v 128]
        wo_d = din("wo", [128, HL * C])
        tri_d = din("tri", [D, D])          # tri[i,j] = 1.0 if i<=j else 0.0
        one_d = din("ones", [D, D])
        bq_d = din("bq", [D, HL], F32)      # pre-scaled by 1/sqrt(D)
        bk_d = din("bk", [D, 1], F32)
        bv_d = din("bv", [D, 1], F32)
        cos_d = din("cost", [D, T])         # cos[0].T
        sin_d = din("sins", [D, T])         # sin[0].T with rows 0:64 negated
        out_d = dram.tile([T, C], BF16, kind="ExternalOutput",
                          name="out", uniquify=False)

        const = ctx.enter_context(tc.tile_pool(name="const", bufs=1))
        xt_s = const.tile([128, NCT * T], BF16, name="xt_s")
        wq_s = const.tile([128, NCT * HL * D], BF16, name="wq_s")
        wkv_s = const.tile([128, NCT * 2 * D], BF16, name="wkv_s")
        wo_s = const.tile([128, HL * C], BF16, name="wo_s")
        bq_s = const.tile([128, HL], F32, name="bq_s")
        bk_s = const.tile([128, 1], F32, name="bk_s")
        bv_s = const.tile([128, 1], F32, name="bv_s")
        cos_s = const.tile([128, T], BF16, name="cos_s")
        sin_s = const.tile([128, T], BF16, name="sin_s")
        tri_s = const.tile([128, 128], BF16, name="tri_s")
        ones_s = const.tile([128, 128], BF16, name="ones_s")

        # Weight chunks on the GpSimd SWDGE queue, x chunks on the Sync
        # queue, small consts on the Scalar queue: the first projection
        # matmul only needs the c=0 chunks, so the PE starts ~4us in
        # instead of waiting ~30us for whole-tensor loads.
        for c in range(NCT):
            nc.gpsimd.dma_start(out=wq_s[:, c * HL * D:(c + 1) * HL * D],
                                in_=wq_d[:, c * HL * D:(c + 1) * HL * D])
            nc.gpsimd.dma_start(out=wkv_s[:, c * 2 * D:(c + 1) * 2 * D],
                                in_=wkv_d[:, c * 2 * D:(c + 1) * 2 * D])
        # consts ride behind the weights; all needed >25us in
        for dst, src in ((bq_s, bq_d), (bk_s, bk_d), (bv_s, bv_d),
                         (cos_s, cos_d), (sin_s, sin_d), (tri_s, tri_d),
                         (ones_s, one_d)):
            nc.gpsimd.dma_start(out=dst[:], in_=src[:])
        # x streams per-chunk on the two HWDGE queues with nothing ahead of
        # them; the 8MB stream is HBM-bound (~23us) either way, but small
        # chunks give the earliest c=0 arrival and per-chunk dependencies.
        for c in range(NCT):
            eng = nc.sync if c % 2 == 0 else nc.scalar
            eng.dma_start(out=xt_s[:, c * T:(c + 1) * T],
                          in_=xt_d[:, c * T:(c + 1) * T])

        act = ctx.enter_context(tc.tile_pool(name="act", bufs=1))
        qr = [act.tile([128, T], BF16, name=f"qr{h}_s") for h in range(HL)]
        kr_s = act.tile([128, T], BF16, name="kr_s")
        vT_s = act.tile([128, T], BF16, name="vT_s")
        v_s = act.tile([128, NK * D], BF16, name="v_s")
        ys = [act.tile([128, T], BF16, name=f"y{h}_s") for h in range(HL)]

        prepool = ctx.enter_context(tc.tile_pool(name="prepool", bufs=3))
        rpool = ctx.enter_context(tc.tile_pool(name="rpool", bufs=3))
        ppool = ctx.enter_context(tc.tile_pool(name="ppool", bufs=4))
        apool = ctx.enter_context(tc.tile_pool(name="apool", bufs=2))
        opool = ctx.enter_context(tc.tile_pool(name="opool", bufs=2))

        # ONE PSUM pool for the whole kernel with manually-assigned bank
        # tags: a proj->attn pool transition would make the attention's
        # first write wait on the ENTIRE projection drain (zone-level
        # overlap dep); per-tag tiles get precise tile-level WAR deps, so
        # attention starts in banks the q-pass freed 15us earlier.
        # Banks: proj q -> 0..3 (2*half+j), k -> 4+2*half, v -> 5+2*half.
        # attn: scores alternate 0/1, yav cycles 2/3/4, sums 5, o_proj 6/7.
        psum = ctx.enter_context(
            tc.tile_pool(name="psum", bufs=1, space="PSUM"))

        def pbank(i, name):
            return psum.tile([128, TQ], F32, tag=f"bank{i}", name=name)

        def rope(dst, pre, t, eng):
            """dst = pre*cos + rot_half(pre)*sin on columns [t*TQ, (t+1)*TQ).

            All operands bf16 in SBUF. The partition-swap copies ride the
            idle Scalar HWDGE queue. eng picks the mul/add engine: DVE when
            it is idle, GpSimd when the result is only needed much later
            (its tensor ops run ~1.3us apiece).
            """
            sl = bass.ts(t, TQ)
            rot = rpool.tile([128, TQ], BF16, tag="rot")
            nc.scalar.dma_start(out=rot[0:64, :], in_=pre[64:128, :])
            nc.scalar.dma_start(out=rot[64:128, :], in_=pre[0:64, :])
            eng.tensor_mul(rot[:], rot[:], sin_s[:, sl])
            tmp = rpool.tile([128, TQ], BF16, tag="rtmp")
            eng.tensor_mul(tmp[:], pre[:], cos_s[:, sl])
            eng.tensor_add(dst, tmp[:], rot[:])

        def proj_pair(tp):
            """Combined QKV projection for pair tp (8 PSUM banks). Used for
            pair 0: its 1.84us/chunk consumption matches the ~1.4-2us/chunk
            HBM supply rate of the x stream (a split q-pass would consume
            2x faster than HBM can feed it)."""
            ps = [[pbank(2 * half + j, f"pp{half}{j}") for j in range(2)] +
                  [pbank(4 + 2 * half, f"ppk{half}"),
                   pbank(5 + 2 * half, f"ppv{half}")]
                  for half in range(2)]
            for c in range(NCT):
                st, sp = (c == 0), (c == NCT - 1)
                xb = c * T + tp * 2 * TQ
                wqc, wkc = c * HL * D, c * 2 * D
                for j, wsl in ((0, wq_s[:, wqc:wqc + D]),
                               (1, wq_s[:, wqc + D:wqc + 2 * D]),
                               (2, wkv_s[:, wkc:wkc + D]),
                               (3, wkv_s[:, wkc + D:wkc + 2 * D])):
                    for half in range(2):
                        nc.tensor.matmul(
                            ps[half][j][:], wsl,
                            xt_s[:, xb + half * TQ:xb + (half + 1) * TQ],
                            start=st, stop=sp)
            for half in range(2):
                t = 2 * tp + half
                sl = bass.ts(t, TQ)
                prek = prepool.tile([128, TQ], BF16, tag="prek")
                nc.scalar.activation(prek[:], ps[half][2][:], Id,
                                     bias=bk_s[:, 0:1])
                rope(kr_s[:, sl], prek, t, nc.vector)
                for j in range(2):
                    pre = prepool.tile([128, TQ], BF16, tag=f"pre{j}")
                    nc.scalar.activation(pre[:], ps[half][j][:], Id,
                                         bias=bq_s[:, j:j + 1], scale=scale)
                    rope(qr[j][:, bass.ts(t, TQ)], pre, t, nc.vector)
                nc.scalar.activation(vT_s[:, sl], ps[half][3][:], Id,
                                     bias=bv_s[:, 0:1])
                for tk in range(4 * t, 4 * t + 4):
                    nc.sync.dma_start_transpose(v_s[:, bass.ts(tk, D)],
                                                vT_s[:, bass.ts(tk, D)])

        def proj_q(tp):
            """Q projection only (banks 0-3). Used for pair 1 when x is
            already resident: splitting q from k/v de-lumps the drain so
            this pass's evictions and RoPE run on ACT/GpSimd while the kv
            pass occupies the PE - and its 4 banks are free again well
            before the attention phase wants them for scores."""
            ps = [[pbank(2 * half + j, f"pq{half}{j}") for j in range(2)]
                  for half in range(2)]
            for c in range(NCT):
                st, sp = (c == 0), (c == NCT - 1)
                xb = c * T + tp * 2 * TQ
                wqc = c * HL * D
                for j in range(2):
                    for half in range(2):
                        nc.tensor.matmul(
                            ps[half][j][:],
                            wq_s[:, wqc + j * D:wqc + (j + 1) * D],
                            xt_s[:, xb + half * TQ:xb + (half + 1) * TQ],
                            start=st, stop=sp)
            for half in range(2):
                t = 2 * tp + half
                for j in range(2):
                    pre = prepool.tile([128, TQ], BF16, tag=f"pre{j}")
                    nc.scalar.activation(pre[:], ps[half][j][:], Id,
                                         bias=bq_s[:, j:j + 1], scale=scale)
                    # GpSimd ropes: slow (~1.3us/op) but these q halves are
                    # first read by attention block 2, ~30us later
                    rope(qr[j][:, bass.ts(t, TQ)], pre, t, nc.gpsimd)

        def proj_kv_mm(tp):
            """K/V projection matmuls (banks 4-7)."""
            ps = [[pbank(4 + 2 * half, f"pk{half}"),
                   pbank(5 + 2 * half, f"pv{half}")] for half in range(2)]
            for c in range(NCT):
                st, sp = (c == 0), (c == NCT - 1)
                xb = c * T + tp * 2 * TQ
                wkc = c * 2 * D
                for j in range(2):
                    for half in range(2):
                        nc.tensor.matmul(
                            ps[half][j][:],
                            wkv_s[:, wkc + j * D:wkc + (j + 1) * D],
                            xt_s[:, xb + half * TQ:xb + (half + 1) * TQ],
                            start=st, stop=sp)
            return ps

        def proj_kv_drain(tp, ps):
            """K/V drain, emitted AFTER the first attention head block so
            the attention pipeline's exp/tri/adds win the ACT/DVE queues.
            half0's k-rope on DVE (attn block 2 needs it soon); half1's on
            GpSimd."""
            for half in range(2):
                t = 2 * tp + half
                sl = bass.ts(t, TQ)
                prek = prepool.tile([128, TQ], BF16, tag="prek")
                nc.scalar.activation(prek[:], ps[half][0][:], Id,
                                     bias=bk_s[:, 0:1])
                rope(kr_s[:, sl], prek, t,
                     nc.vector if half == 0 else nc.gpsimd)
                # v eviction on DVE: keeps ACT clear for the attention exps
                nc.vector.tensor_scalar(vT_s[:, sl], ps[half][1][:],
                                        1.0, bv_s[:, 0:1],
                                        mybir.AluOpType.mult,
                                        mybir.AluOpType.add)
                for tk in range(4 * t, 4 * t + 4):
                    nc.sync.dma_start_transpose(v_s[:, bass.ts(tk, D)],
                                                vT_s[:, bass.ts(tk, D)])

        yav_cycle = [0]

        def attn_head(tq, h):
            """Scores + AV for one head of query block tq."""
            ntk = 4 * tq + 4
            yp = pbank(2 + yav_cycle[0] % 3, f"yav{tq}{h}")
            yav_cycle[0] += 1
            # denominator: two accumulator chains so the tile adds can be
            # split between DVE (fast) and GpSimd (slower but idle)
            accD = apool.tile([128, TQ], BF16, tag="accD", bufs=3)
            accG = apool.tile([128, TQ], BF16, tag="accG", bufs=3)
            nd = ng = 0
            # software-pipelined: score matmul+exp one tile ahead of the
            # consuming AV matmul so PE never waits on ACT
            pts = {}
            for tk in range(ntk + 1):
                if tk < ntk:
                    r = max(tk * D - tq * TQ, 0)  # masked col prefix
                    sp_ = pbank(tk % 2, f"s{tq}{h}{tk}")
                    nc.tensor.matmul(
                        sp_[:, r:], kr_s[:, bass.ts(tk, D)],
                        qr[h][:, tq * TQ + r:(tq + 1) * TQ],
                        start=True, stop=True)
                    pt = ppool.tile([128, TQ], BF16, tag="p")
                    nc.scalar.activation(pt[:, r:], sp_[:, r:], Exp)
                    if tk * D >= tq * TQ:  # diagonal: causal mask
                        nc.vector.tensor_mul(pt[:, r:r + D],
                                             pt[:, r:r + D], tri_s[:])
                    # GpSimd only helps for tq>=2: earlier it is still
                    # working through pair 1's RoPE backlog
                    if tq < 2 or tk % 3 == 0:
                        eng, acc = nc.vector, accD
                        nd += 1
                        first = nd == 1
                    else:
                        eng, acc = nc.gpsimd, accG
                        ng += 1
                        first = ng == 1
                    if first and r > 0:
                        eng.memset(acc[:, 0:r], 0.0)
                    if first:
                        eng.tensor_copy(acc[:, r:], pt[:, r:])
                    else:
                        eng.tensor_add(acc[:, r:], acc[:, r:], pt[:, r:])
                    pts[tk] = (pt, r)
                if tk >= 1:
                    pt, r = pts.pop(tk - 1)
                    nc.tensor.matmul(yp[:, r:],
                                     v_s[:, bass.ts(tk - 1, D)],
                                     pt[:, r:], start=(tk - 1 == 0),
                                     stop=(tk - 1 == ntk - 1))
            return (yp, accD, accG, ng)

        def attn_fin(tq, h, fin):
            """Denominator matmul + y normalization for (tq, h). Emitted
            1-2 head-blocks behind attn_head so the acc chains and the
            recip/ymul are never on the PE critical path."""
            yp, accD, accG, ng = fin
            sump = pbank(5, f"sum{tq}{h}")
            nc.tensor.matmul(sump[:], ones_s[:], accD[:],
                             start=True, stop=(ng == 0))
            if ng:
                nc.tensor.matmul(sump[:], ones_s[:], accG[:],
                                 start=False, stop=True)
            rec = rpool.tile([128, TQ], F32, tag="rec")
            nc.vector.reciprocal_approx_fast(rec[:], sump[:])
            nc.vector.tensor_mul(ys[h][:, bass.ts(tq, TQ)], yp[:], rec[:])

        def oproj(tq):
            """o_proj partial for the 4 row-tiles of query block tq.
            Evictions split ACT/DVE; two DMAs per row-tile."""
            for tt in range(4):
                t = 4 * tq + tt
                wide = opool.tile([128, C], BF16, tag="oev")
                for n in range(NT):
                    op_ = pbank(6 + n % 2, f"o{t}{n}")
                    for h in range(HL):
                        nc.tensor.matmul(
                            op_[:], ys[h][:, bass.ts(t, D)],
                            wo_s[:, h * C + n * TQ:h * C + (n + 1) * TQ],
                            start=(h == 0), stop=(h == HL - 1))
                    # evictions split ACT/DVE 50/50 to balance both
                    # engines against exp/adds; GpSimd cannot read PSUM
                    if (t * NT + n) % 2 == 0:
                        nc.scalar.activation(wide[:, bass.ts(n, TQ)],
                                             op_[:], Id)
                    else:
                        nc.vector.tensor_copy(wide[:, bass.ts(n, TQ)],
                                              op_[:])
                    if n == 1:  # first half done -> overlap DMA with n=2,3
                        nc.sync.dma_start(
                            out=out_d[bass.ts(t, D), 0:2 * TQ],
                            in_=wide[:, 0:2 * TQ])
                nc.sync.dma_start(out=out_d[bass.ts(t, D), 2 * TQ:],
                                  in_=wide[:, 2 * TQ:])

        # Pipelined schedule: the pair-1 k/v drain is emitted AFTER the
        # first attention head so the attention pipeline's exp/tri/adds
        # win the ACT/DVE queues; fin(tq,h) trails its attn_head by 1-2
        # head blocks (acc chains/recip/ymul finish off the critical
        # path); oproj(tq) trails fin(tq,h1).
        proj_pair(0)
        nc.gpsimd.dma_start(out=wo_s[:], in_=wo_d[:])
        proj_q(1)
        ps_kv = proj_kv_mm(1)
        f = {}
        f[0, 0] = attn_head(0, 0)
        proj_kv_drain(1, ps_kv)
        f[0, 1] = attn_head(0, 1)
        attn_fin(0, 0, f[0, 0])
        f[1, 0] = attn_head(1, 0)
        attn_fin(0, 1, f[0, 1])
        f[1, 1] = attn_head(1, 1)
        attn_fin(1, 0, f[1, 0])
        oproj(0)
        f[2, 0] = attn_head(2, 0)
        attn_fin(1, 1, f[1, 1])
        f[2, 1] = attn_head(2, 1)
        attn_fin(2, 0, f[2, 0])
        oproj(1)
        f[3, 0] = attn_head(3, 0)
        attn_fin(2, 1, f[2, 1])
        f[3, 1] = attn_head(3, 1)
        attn_fin(3, 0, f[3, 0])
        oproj(2)
        attn_fin(3, 1, f[3, 1])
        oproj(3)
    nc.compile()
    return nc


def _get_nc():
    if "nc" not in _CACHE:
        _CACHE["nc"] = _build()
    return _CACHE["nc"]


def _prep_inputs(x, cos, sin, Wq, bq, Wk, bk, Wv, bv, Wo):
    f = np.float32
    bf = ml_dtypes.bfloat16
    xT = np.asarray(x[0].T, dtype=np.float32)
    # partition-major tiling to match the SBUF layout: one contiguous
    # [128, k*2048] slab per contraction chunk
    xT = np.ascontiguousarray(
        xT.reshape(NCT, 128, T).transpose(1, 0, 2).reshape(128, -1), dtype=bf)
    cosT = np.ascontiguousarray(cos[0].T, dtype=bf)
    sinT = np.asarray(sin[0].T, dtype=f)
    sins = np.concatenate([-sinT[:64], sinT[64:]], axis=0)
    sins = np.ascontiguousarray(sins, dtype=bf)
    idx = np.arange(D)
    tri = (idx[:, None] <= idx[None, :]).astype(bf)
    ones = np.ones((D, D), dtype=bf)
    scale = np.float32(1.0 / math.sqrt(D))
    in_maps = []

    def ptile(a):
        """[K*128, N] -> partition-major [128, K*N] matching the SBUF tiles."""
        k = a.shape[0] // 128
        return a.reshape(k, 128, a.shape[1]).transpose(1, 0, 2).reshape(128, -1)

    for m in range(NCORES):
        g = m // 2
        wq_m = np.ascontiguousarray(
            ptile(Wq[m * 256:(m + 1) * 256, :].T.astype(f)), dtype=bf)
        wk_m = ptile(Wk[g * 128:(g + 1) * 128, :].T.astype(f))
        wv_m = ptile(Wv[g * 128:(g + 1) * 128, :].T.astype(f))
        # interleave [k | v] per contraction chunk
        wkv_m = np.empty((128, NCT * 2 * D), dtype=f)
        for c in range(NCT):
            wkv_m[:, c * 2 * D:c * 2 * D + D] = wk_m[:, c * D:(c + 1) * D]
            wkv_m[:, c * 2 * D + D:(c + 1) * 2 * D] = wv_m[:, c * D:(c + 1) * D]
        wkv_m = np.ascontiguousarray(wkv_m, dtype=bf)
        wo_m = np.ascontiguousarray(
            ptile(Wo[:, m * 256:(m + 1) * 256].T.astype(f)), dtype=bf)
        bq_m = np.ascontiguousarray(
            (bq[m * 256:(m + 1) * 256] * scale).reshape(HL, D).T, dtype=f)
        bk_m = np.ascontiguousarray(bk[g * 128:(g + 1) * 128].reshape(D, 1),
                                    dtype=f)
        bv_m = np.ascontiguousarray(bv[g * 128:(g + 1) * 128].reshape(D, 1),
                                    dtype=f)
        in_maps.append({
            "xt": xT, "wq": wq_m, "wkv": wkv_m, "wo": wo_m,
            "bq": bq_m, "bk": bk_m, "bv": bv_m,
            "cost": cosT, "sins": sins, "tri": tri, "ones": ones,
        })
    return in_maps


def kernel(x, cos, sin, Wq, bq, Wk, bk, Wv, bv, Wo, _trace=False):
    x, cos, sin = np.asarray(x), np.asarray(cos), np.asarray(sin)
    Wq, bq = np.asarray(Wq), np.asarray(bq)
    Wk, bk = np.asarray(Wk), np.asarray(bk)
    Wv, bv = np.asarray(Wv), np.asarray(bv)
    Wo = np.asarray(Wo)
    nc = _get_nc()
    in_maps = _prep_inputs(x, cos, sin, Wq, bq, Wk, bk, Wv, bv, Wo)
    res = run_bass_kernel_spmd(nc, in_maps, core_ids=list(range(NCORES)),
                               trace=_trace)
    out = res.results[0]["out"].astype(np.float64)
    for m in range(1, NCORES):
        out += res.results[m]["out"]
    out = out.astype(np.float32).reshape(B, T, C)
    if _trace:
        _CACHE["last_result"] = res
    return out
